# revision 10
# baseline (speedup 1.0000x reference)
"""Trainium2 Bass kernel for nn_Decoder_31164282700387.

Tensor-parallel over hidden/feature dims across 8 cores, feat-major
([feature, batch]) layouts so batch (256) is the matmul moving dim. The
constant h_i terms of the LSTM-gate / W4 / W7 concats are precomputed into
biases (fp32r matmuls). Scan matmuls run in bf16 (weights + gathered
activations; fp32 PSUM accumulate), pointwise math in fp32. Per step:
gates -> LSTM pointwise -> AG(h||c) -> W1 -> AG(y1) -> W2 -> AG(y2) -> W3
-> AG(y) -> W4 -> AG(hz) -> W5|6 -> z -> AG(z); the prior branch (W7)
fills PE gaps, W8/W9 + z_p run post-scan after one AG of stashed hz_p.
"""
import os
import numpy as np

import concourse.bass as bass
import concourse.tile as tile
import concourse.mybir as mybir
from concourse import bacc
from concourse.bass_utils import run_bass_kernel_spmd
from concourse.masks import make_identity

F32 = mybir.dt.float32
F32R = mybir.dt.float32r
BF16 = mybir.dt.bfloat16
AF = mybir.ActivationFunctionType
ALU = mybir.AluOpType
AX = mybir.AxisListType

B, H, F, Z = 256, 2048, 1024, 512
T, TS, PREV = 16, 32, 16
NCORE = 8
HS, FS, ZS = H // NCORE, F // NCORE, Z // NCORE  # 256, 128, 64
RG = [list(range(NCORE))]
REPS = int(os.environ.get("KERNEL_REPS", "1"))
MMDT = {"bf16": BF16, "f32r": F32R}[os.environ.get("KERNEL_MMDT", "bf16")]

KH, KF, KZ = H // 128, F // 128, Z // 128  # 16, 8, 4


def _build():
    nc = bacc.Bacc(None, target_bir_lowering=False, num_devices=NCORE)

    def inp(name, shape, dt=F32):
        return nc.declare_dram_parameter(name, list(shape), dt, isOutput=False)

    def outp(name, shape, dt=F32):
        return nc.declare_dram_parameter(name, list(shape), dt, isOutput=True)

    # ---- inputs (per-core data differs, graph identical) ----
    hiT = inp("hiT", [H, B], F32R)
    t_nat = inp("t_nat", [B, TS])
    scal = inp("scal", [1, 4])
    wkzy = inp("wkzy", [Z + F, 1024], MMDT)
    wkh = inp("wkh", [H, 1024], F32R)
    wr = inp("wr", [H, 1024], MMDT)
    blstm = inp("blstm", [128, 8])
    w1 = inp("w1", [H, FS], MMDT)
    b1 = inp("b1", [FS, 1])
    w2 = inp("w2", [F, FS], MMDT)
    b2 = inp("b2", [FS, 1])
    w3 = inp("w3", [F, FS], MMDT)
    b3 = inp("b3", [FS, 1])
    w4h = inp("w4h", [H, ZS], F32R)
    w4s = inp("w4s", [H + 2 * F, ZS], MMDT)
    b4 = inp("b4", [ZS, 1])
    w5 = inp("w5", [Z, ZS], MMDT)
    b5 = inp("b5", [ZS, 1])
    w6 = inp("w6", [Z, ZS], MMDT)
    b6 = inp("b6", [ZS, 1])
    w7h = inp("w7h", [H, ZS], F32R)
    w7s = inp("w7s", [H + F, ZS], MMDT)
    b7 = inp("b7", [ZS, 1])
    w8 = inp("w8", [Z, ZS], MMDT)
    b8 = inp("b8", [ZS, 1])
    w9 = inp("w9", [Z, ZS], MMDT)
    b9 = inp("b9", [ZS, 1])
    nzi = inp("nzi", [T, ZS, B])
    nzp = inp("nzp", [T, ZS, B])

    # ---- outputs (per-core slices, feat-major; host re-assembles) ----
    o_y = outp("o_y", [T, FS, B])
    o_mean = outp("o_mean", [T, ZS, B])
    o_logv = outp("o_logv", [T, ZS, B])
    o_z = outp("o_z", [T, ZS, B])
    o_zp = outp("o_zp", [T, ZS, B])
    o_t = outp("o_t", [T, B])
    o_te = outp("o_te", [T, B])

    with tile.TileContext(nc) as tc:
        import contextlib
        stack = contextlib.ExitStack()
        weights = stack.enter_context(tc.tile_pool(name="weights", bufs=1))
        consts = stack.enter_context(tc.tile_pool(name="consts", bufs=1))
        dram = stack.enter_context(tc.tile_pool(name="dram", bufs=1, space="DRAM"))
        ps = stack.enter_context(tc.tile_pool(name="ps", bufs=3, space="PSUM"))
        ps1 = stack.enter_context(tc.tile_pool(name="ps1", bufs=2, space="PSUM"))

        # ======== resident weights ========
        def wload(name, src, ktiles, mwid):
            t_ = weights.tile([128, ktiles, mwid], MMDT, name=name)
            nc.sync.dma_start(t_[:], src.rearrange("(k p) m -> p k m", p=128))
            return t_

        wr_s = wload("wr_s", wr, KH, 1024)
        wkzy_s = wload("wkzy_s", wkzy, KZ + KF, 1024)
        w1_s = wload("w1_s", w1, KH, FS)
        w2_s = wload("w2_s", w2, KF, FS)
        w3_s = wload("w3_s", w3, KF, FS)
        w4s_s = wload("w4s_s", w4s, KH + 2 * KF, ZS)
        w7s_s = wload("w7s_s", w7s, KH + KF, ZS)
        w56_s = weights.tile([128, KZ, 2 * ZS], MMDT)
        nc.sync.dma_start(w56_s[:, :, :ZS], w5.rearrange("(k p) m -> p k m", p=128))
        nc.sync.dma_start(w56_s[:, :, ZS:], w6.rearrange("(k p) m -> p k m", p=128))
        w89_s = weights.tile([128, KZ, 2 * ZS], MMDT)
        nc.sync.dma_start(w89_s[:, :, :ZS], w8.rearrange("(k p) m -> p k m", p=128))
        nc.sync.dma_start(w89_s[:, :, ZS:], w9.rearrange("(k p) m -> p k m", p=128))

        blstm_s = consts.tile([128, 8], F32)
        nc.sync.dma_start(blstm_s[:], blstm[:, :])
        bvec = consts.tile([128, 4], F32)  # b1 | b56 | b89 | b4b7
        nc.sync.dma_start(bvec[:, 0:1], b1[:, :])
        nc.sync.dma_start(bvec[0:ZS, 1:2], b5[:, :])
        nc.sync.dma_start(bvec[ZS:128, 1:2], b6[:, :])
        nc.sync.dma_start(bvec[0:ZS, 2:3], b8[:, :])
        nc.sync.dma_start(bvec[ZS:128, 2:3], b9[:, :])
        nc.sync.dma_start(bvec[0:ZS, 3:4], b4[:, :])
        nc.sync.dma_start(bvec[ZS:128, 3:4], b7[:, :])
        bvec23 = consts.tile([128, 2], F32)  # b2 | b3
        nc.sync.dma_start(bvec23[:, 0:1], b2[:, :])
        nc.sync.dma_start(bvec23[:, 1:2], b3[:, :])

        gbias = consts.tile([128, 8, B], F32)     # (h_i @ Wk_h + b)^T slice
        bias4 = consts.tile([ZS, B], F32)
        bias7 = consts.tile([ZS, B], F32)
        lam_bc_d = dram.tile([T, 128, B], F32, name="lam_bc_d")

        # ======== precompute phase (scoped pools; fp32r) ========
        with (
            tc.tile_pool(name="pre", bufs=3) as pre,
            tc.tile_pool(name="pre1", bufs=1) as pre1,
            tc.tile_pool(name="ps_pre", bufs=1, space="PSUM") as ps_pre,
        ):
            ident = pre1.tile([128, 128], F32)
            make_identity(nc, ident[:])
            ones_f = pre1.tile([1, 128], F32)
            nc.vector.memset(ones_f[:], 1.0)
            ones_r = pre1.tile([1, 128], F32R)
            nc.vector.tensor_copy(ones_r[:], ones_f[:])
            sel_np = np.zeros((T, T, 128), np.float32)
            for j in range(T):
                sel_np[j, j, :] = 1.0
            sel_d = nc.inline_tensor(sel_np, name="sel_d")
            sel = pre1.tile([T, T, 128], F32R)
            nc.sync.dma_start(sel[:], sel_d.ap().bitcast(F32R))

            # scalars: ab = alpha*beta, base, htw = tw*H -> [128,1] bcast
            s4 = pre1.tile([1, 4], F32)
            nc.sync.dma_start(s4[:], scal[:, :])
            sv = pre1.tile([1, 4], F32R)
            nc.vector.tensor_tensor(sv[:, 0:1], s4[:, 0:1], s4[:, 1:2], ALU.mult)
            nc.vector.tensor_copy(sv[:, 1:2], s4[:, 2:3])
            nc.vector.tensor_scalar_mul(sv[:, 2:3], s4[:, 3:4], float(H))
            nc.vector.tensor_copy(sv[:, 3:4], s4[:, 3:4])
            ps_bc = ps_pre.tile([128, 4], F32, name="ps_bc", tag="psQ")
            nc.tensor.matmul(ps_bc[:], ones_r[:], sv[:], start=True, stop=True)
            scbc = pre1.tile([128, 3], F32)  # ab | base | htw
            nc.vector.tensor_copy(scbc[:], ps_bc[:, 0:3])

            hi_s = pre1.tile([128, KH, B], F32R)
            nc.sync.dma_start(hi_s[:], hiT.rearrange("(k p) b -> p k b", p=128))

            # gbias = (h_i @ Wk_h)^T + b_lstm
            wkh_v = wkh.rearrange("(k p) m -> p k m", p=128)
            wkh_tiles = []
            for k in range(KH):
                wt = pre1.tile([128, 1024], F32R, tag=f"wkh_{k}", name=f"wkh_{k}")
                nc.sync.dma_start(wt[:], wkh_v[:, k])
                wkh_tiles.append(wt)
            for m in range(8):
                pg = ps_pre.tile([128, B], F32, name="pg", tag="psP")
                for k in range(KH):
                    nc.tensor.matmul(pg[:], wkh_tiles[k][:, m * 128:(m + 1) * 128],
                                     hi_s[:, k], start=(k == 0), stop=(k == KH - 1))
                nc.scalar.activation(gbias[:, m], pg[:], AF.Identity,
                                     bias=blstm_s[:, m:m + 1])

            # bias4 / bias7
            for bias_t, w_t in ((bias4, w4h), (bias7, w7h)):
                pb = ps_pre.tile([128, B], F32, name="pb", tag="psP")[0:ZS]
                wv = w_t.rearrange("(k p) m -> p k m", p=128)
                for k in range(KH):
                    wt = pre.tile([128, ZS], F32R, tag="wh_st")
                    nc.sync.dma_start(wt[:], wv[:, k])
                    nc.tensor.matmul(pb[:], wt[:], hi_s[:, k],
                                     start=(k == 0), stop=(k == KH - 1))
                brow = bvec[0:ZS, 3:4] if bias_t is bias4 else bvec[ZS:128, 3:4]
                nc.scalar.activation(bias_t[:], pb[:], AF.Identity, bias=brow)

            # ---- Hawkes intensity (batch-major halves) ----
            curT = pre1.tile([T, B], F32)
            teT = pre1.tile([T, B], F32)
            lamT = pre1.tile([T, B], F32R)
            for bh in range(2):
                tt = pre.tile([128, TS], F32, tag="tt")
                nc.sync.dma_start(tt[:], t_nat[bh * 128:(bh + 1) * 128, :])
                E = pre.tile([128, TS], F32, tag="E")
                nc.scalar.activation(E[:], tt[:], AF.Exp)
                Rm = pre.tile([128, T], F32, tag="Rm")
                nc.scalar.activation(Rm[:], tt[:, PREV:TS], AF.Exp, scale=-1.0)
                S = pre.tile([128, T], F32, tag="S")
                for j in range(T):
                    nc.vector.reduce_sum(S[:, j:j + 1], E[:, 0:PREV + j], axis=AX.X)
                trig = pre.tile([128, T], F32, tag="trig")
                nc.vector.tensor_mul(trig[:], Rm[:], S[:])
                lam_b = pre.tile([128, T], F32, tag="lam_b")
                nc.vector.tensor_scalar(lam_b[:], trig[:], scbc[:, 0:1],
                                        scbc[:, 1:2], ALU.mult, ALU.add)
                xs = pre.tile([128, T], F32, tag="xs")
                nc.vector.tensor_scalar_mul(xs[:], lam_b[:], scbc[:, 2:3])
                ex = pre.tile([128, T], F32, tag="ex")
                nc.scalar.activation(ex[:], xs[:], AF.Exp)
                ex1 = pre.tile([128, T], F32, tag="ex1")
                nc.vector.tensor_scalar_add(ex1[:], ex[:], 1.0)
                delta = pre.tile([128, T], F32, tag="delta")
                nc.scalar.activation(delta[:], ex1[:], AF.Ln)
                te_b = pre.tile([128, T], F32, tag="te_b")
                nc.vector.tensor_add(te_b[:], tt[:, PREV:TS], delta[:])
                for src, dst in ((lam_b[:], lamT), (tt[:, PREV:TS], curT),
                                 (te_b[:], teT)):
                    pt = ps_pre.tile([T, 128], F32, name="pt", tag="psQ")
                    nc.tensor.transpose(pt[:], src, ident[:])
                    nc.vector.tensor_copy(dst[:, bh * 128:(bh + 1) * 128], pt[:])
            nc.sync.dma_start(o_t[:, :], curT[:])
            nc.sync.dma_start(o_te[:, :], teT[:])
            # lam broadcast rows -> DRAM [T, 128, B]
            for j in range(T):
                pl = ps_pre.tile([128, B], F32, name="pl", tag="psP")
                nc.tensor.matmul(pl[:], sel[:, j, :], lamT[:, :],
                                 start=True, stop=True)
                lam_sb = pre.tile([128, B], F32, tag="lam_sb")
                nc.vector.tensor_copy(lam_sb[:], pl[:])
                nc.sync.dma_start(lam_bc_d[j], lam_sb[:])

        # ======== scan ========
        state = stack.enter_context(tc.tile_pool(name="state", bufs=1))
        ytp = stack.enter_context(tc.tile_pool(name="ytp", bufs=2))
        tmp = stack.enter_context(tc.tile_pool(name="tmp", bufs=2))
        tmp1 = stack.enter_context(tc.tile_pool(name="tmp1", bufs=1))
        dram2 = stack.enter_context(tc.tile_pool(name="dram2", bufs=2, space="DRAM"))

        hT = state.tile([128, KH, B], MMDT)
        cT = state.tile([128, KH, B], MMDT)
        zT = state.tile([128, KZ, B], MMDT)
        c_state = state.tile([128, 2, B], F32)
        hzp_dump = dram.tile([T, ZS, B], MMDT, name="hzp_dump")

        for rep in range(REPS):
            yT_prev = None
            for j in range(T):
                # ---- gates ----
                sig_i, sig_f, tanh_g, sig_o = [], [], [], []
                acts = [(AF.Sigmoid, sig_i), (AF.Sigmoid, sig_f),
                        (AF.Tanh, tanh_g), (AF.Sigmoid, sig_o)]
                for m in range(8):
                    func, lst = acts[m // 2]
                    if j == 0:
                        g_in = gbias[:, m]
                    else:
                        pg = ps.tile([128, B], F32, name="ps_g", tag="psA")
                        for k in range(KH):
                            nc.tensor.matmul(pg[:], wr_s[:, k, m * 128:(m + 1) * 128],
                                             hT[:, k], start=(k == 0), stop=False)
                        for k in range(KF):
                            nc.tensor.matmul(pg[:], wkzy_s[:, KZ + k, m * 128:(m + 1) * 128],
                                             yT_prev[:, k], start=False, stop=False)
                        for k in range(KZ):
                            nc.tensor.matmul(pg[:], wkzy_s[:, k, m * 128:(m + 1) * 128],
                                             zT[:, k], start=False, stop=(k == KZ - 1))
                        g_in = tmp.tile([128, B], F32, tag="g_in")
                        nc.vector.tensor_add(g_in[:], pg[:], gbias[:, m])
                        g_in = g_in[:]
                    g_out = tmp.tile([128, B], F32, tag=f"g_out{m}", bufs=1,
                                     name=f"g_out{m}")
                    nc.scalar.activation(g_out[:], g_in, func)
                    lst.append(g_out)

                # ---- cell/hidden pointwise ----
                lam_bc = tmp.tile([128, B], F32, tag="lam_bc")
                nc.sync.dma_start(lam_bc[:], lam_bc_d[j])
                hc_in = dram2.tile([4, 128, B], MMDT, tag="hc_in")
                c_sl = tmp1.tile([128, 2, B], F32, tag="c_sl")
                h_sl = tmp1.tile([128, 2, B], F32, tag="h_sl")
                hc_bf = tmp1.tile([128, 4, B], MMDT, tag="hc_bf")
                for t_ in range(2):
                    ig = tmp.tile([128, B], F32, tag="ig", bufs=1)
                    nc.vector.tensor_mul(ig[:], sig_i[t_][:], tanh_g[t_][:])
                    if j == 0:
                        nc.vector.tensor_copy(c_sl[:, t_], ig[:])
                    else:
                        fc = tmp.tile([128, B], F32, tag="fc", bufs=1)
                        nc.vector.tensor_mul(fc[:], sig_f[t_][:], c_state[:, t_])
                        nc.vector.tensor_add(c_sl[:, t_], fc[:], ig[:])
                    tc_ = tmp.tile([128, B], F32, tag="tc_", bufs=1)
                    nc.scalar.activation(tc_[:], c_sl[:, t_], AF.Tanh)
                    nc.vector.tensor_mul(h_sl[:, t_], sig_o[t_][:], tc_[:])
                    nc.vector.tensor_mul(c_state[:, t_], c_sl[:, t_], lam_bc[:])
                    nc.vector.tensor_copy(hc_bf[:, t_], h_sl[:, t_])
                    nc.vector.tensor_copy(hc_bf[:, 2 + t_], c_sl[:, t_])
                    nc.sync.dma_start(hc_in[t_], hc_bf[:, t_])
                    nc.sync.dma_start(hc_in[2 + t_], hc_bf[:, 2 + t_])

                # ---- AG(h || c) ----
                hc_out = dram2.tile([NCORE, 4, 128, B], MMDT, tag="hc_out",
                                    addr_space="Shared")
                nc.gpsimd.collective_compute(
                    "AllGather", ALU.bypass, replica_groups=RG,
                    ins=[hc_in.opt()], outs=[hc_out.opt()])
                for k in range(KH):
                    nc.sync.dma_start(hT[:, k], hc_out[k // 2, k % 2])
                    nc.sync.dma_start(cT[:, k], hc_out[k // 2, 2 + (k % 2)])

                # ---- W1 -> AG(y1) -> W2 -> AG(y2) -> W3 -> AG(y) ----
                def dense_ag(w_tile, kt, rhs_tiles, bias_ap, tag, out_f32=None):
                    p = ps.tile([128, B], F32, name=f"ps_{tag}", tag="psA")
                    for k in range(kt):
                        nc.tensor.matmul(p[:], w_tile[:, k], rhs_tiles[k],
                                         start=(k == 0), stop=(k == kt - 1))
                    sl = tmp1.tile([128, B], MMDT, tag="sl_y", bufs=2,
                                   name=f"sl_{tag}")
                    if out_f32 is not None:
                        nc.scalar.activation(out_f32[:], p[:], AF.Relu, bias=bias_ap)
                        nc.vector.tensor_copy(sl[:], out_f32[:])
                    else:
                        nc.scalar.activation(sl[:], p[:], AF.Relu, bias=bias_ap)
                    bounce_in = dram2.tile([128, B], MMDT, tag="b_in", bufs=2,
                                           name=f"bin_{tag}")
                    nc.sync.dma_start(bounce_in[:], sl[:])
                    bounce_out = dram2.tile([NCORE, 128, B], MMDT, tag="b_out",
                                            bufs=2, name=f"bout_{tag}",
                                            addr_space="Shared")
                    nc.gpsimd.collective_compute(
                        "AllGather", ALU.bypass, replica_groups=RG,
                        ins=[bounce_in.opt()], outs=[bounce_out.opt()])
                    pool_ = ytp if tag == "y3" else tmp1
                    gath = pool_.tile([128, KF, B], MMDT, tag="ygath", bufs=2,
                                      name=f"g_{tag}")
                    for k in range(KF):
                        nc.sync.dma_start(gath[:, k], bounce_out[k])
                    return gath

                y1g = dense_ag(w1_s, KH, [hT[:, k] for k in range(KH)],
                               bvec[:, 0:1], "y1")
                y2g = dense_ag(w2_s, KF, [y1g[:, k] for k in range(KF)],
                               bvec23[:, 0:1], "y2")
                y_f32 = tmp.tile([128, B], F32, tag="y_f32")
                yT_cur = dense_ag(w3_s, KF, [y2g[:, k] for k in range(KF)],
                                  bvec23[:, 1:2], "y3", out_f32=y_f32)
                nc.sync.dma_start(o_y[j], y_f32[:])

                # ---- W4 -> AG(hz) ----
                p4 = ps.tile([128, B], F32, name="ps_w4", tag="psA")[0:ZS]
                nk4 = KH + KF + (KF if j > 0 else 0)
                ki = 0
                for k in range(KH):
                    nc.tensor.matmul(p4[:], w4s_s[:, k, :], cT[:, k],
                                     start=(ki == 0), stop=(ki == nk4 - 1)); ki += 1
                for k in range(KF):
                    nc.tensor.matmul(p4[:], w4s_s[:, KH + k, :], yT_cur[:, k],
                                     start=(ki == 0), stop=(ki == nk4 - 1)); ki += 1
                if j > 0:
                    for k in range(KF):
                        nc.tensor.matmul(p4[:], w4s_s[:, KH + KF + k, :],
                                         yT_prev[:, k],
                                         start=(ki == 0), stop=(ki == nk4 - 1))
                        ki += 1
                hz4 = tmp.tile([ZS, B], F32, tag="hz4", bufs=1)
                nc.vector.tensor_add(hz4[:], p4[:], bias4[:])
                hz_sl = tmp1.tile([ZS, B], MMDT, tag="hz_sl", bufs=2)
                nc.scalar.activation(hz_sl[:], hz4[:], AF.Relu)
                hz_in = dram2.tile([ZS, B], MMDT, tag="hz_in")
                nc.sync.dma_start(hz_in[:], hz_sl[:])
                hz_out = dram2.tile([NCORE, ZS, B], MMDT, tag="hz_out",
                                    addr_space="Shared")
                nc.gpsimd.collective_compute(
                    "AllGather", ALU.bypass, replica_groups=RG,
                    ins=[hz_in.opt()], outs=[hz_out.opt()])
                hzT = tmp1.tile([128, KZ, B], MMDT, tag="hzT")
                for k in range(KZ):
                    nc.sync.dma_start(hzT[:64, k], hz_out[2 * k])
                    nc.sync.dma_start(hzT[64:128, k], hz_out[2 * k + 1])

                # ---- W5 | W6 -> mean, logv, z ----
                p56 = ps.tile([128, B], F32, name="ps_w56", tag="psA")
                for k in range(KZ):
                    nc.tensor.matmul(p56[:], w56_s[:, k], hzT[:, k],
                                     start=(k == 0), stop=(k == KZ - 1))
                ml = tmp.tile([128, B], F32, tag="ml")
                nc.scalar.activation(ml[:], p56[:], AF.Relu, bias=bvec[:, 1:2])
                nc.sync.dma_start(o_mean[j], ml[0:ZS, :])
                nc.sync.dma_start(o_logv[j], ml[ZS:128, :])
                logv_lo = tmp.tile([ZS, B], F32, tag="logv_lo", bufs=1)
                nc.sync.dma_start(logv_lo[:], ml[ZS:128, :])
                std = tmp.tile([ZS, B], F32, tag="std", bufs=1)
                nc.scalar.activation(std[:], logv_lo[:], AF.Exp, scale=0.5)
                nz_t = tmp.tile([ZS, B], F32, tag="nz_t")
                nc.sync.dma_start(nz_t[:], nzi[j])
                nstd = tmp.tile([ZS, B], F32, tag="nstd", bufs=1)
                nc.vector.tensor_mul(nstd[:], nz_t[:], std[:])
                z_f32 = tmp.tile([ZS, B], F32, tag="z_f32")
                nc.vector.tensor_add(z_f32[:], ml[0:ZS, :], nstd[:])
                nc.sync.dma_start(o_z[j], z_f32[:])
                z_sl = tmp1.tile([ZS, B], MMDT, tag="z_sl", bufs=2)
                nc.vector.tensor_copy(z_sl[:], z_f32[:])

                # ---- AG(z) ----
                z_in = dram2.tile([ZS, B], MMDT, tag="z_in")
                nc.sync.dma_start(z_in[:], z_sl[:])
                z_out = dram2.tile([NCORE, ZS, B], MMDT, tag="z_out",
                                   addr_space="Shared")
                nc.gpsimd.collective_compute(
                    "AllGather", ALU.bypass, replica_groups=RG,
                    ins=[z_in.opt()], outs=[z_out.opt()])
                for k in range(KZ):
                    nc.sync.dma_start(zT[:64, k], z_out[2 * k])
                    nc.sync.dma_start(zT[64:128, k], z_out[2 * k + 1])

                # ---- W7 (prior branch, fills PE gaps) ----
                p7 = ps1.tile([128, B], F32, name="ps_w7", tag="psB")[0:ZS]
                nk7 = KH + (KF if j > 0 else 0)
                ki = 0
                for k in range(KH):
                    nc.tensor.matmul(p7[:], w7s_s[:, k, :], cT[:, k],
                                     start=(ki == 0), stop=(ki == nk7 - 1)); ki += 1
                if j > 0:
                    for k in range(KF):
                        nc.tensor.matmul(p7[:], w7s_s[:, KH + k, :], yT_prev[:, k],
                                         start=(ki == 0), stop=(ki == nk7 - 1))
                        ki += 1
                hz7 = tmp.tile([ZS, B], F32, tag="hz7", bufs=1)
                nc.vector.tensor_add(hz7[:], p7[:], bias7[:])
                hzp_sl = tmp.tile([ZS, B], MMDT, tag="hzp_sl")
                nc.scalar.activation(hzp_sl[:], hz7[:], AF.Relu)
                nc.sync.dma_start(hzp_dump[j], hzp_sl[:])

                yT_prev = yT_cur

        # ======== deferred prior tail: W8/W9 + z_p ========
        with (
            tc.tile_pool(name="dtmp", bufs=3) as dtmp,
            tc.tile_pool(name="ddram", bufs=1, space="DRAM") as ddram,
            tc.tile_pool(name="ps_d", bufs=2, space="PSUM") as ps_d,
        ):
            hzp_all = ddram.tile([NCORE, T, ZS, B], MMDT, addr_space="Shared",
                                 name="hzp_all")
            nc.gpsimd.collective_compute(
                "AllGather", ALU.bypass, replica_groups=RG,
                ins=[hzp_dump.opt()], outs=[hzp_all.opt()])
            for j in range(T):
                hzpT = dtmp.tile([128, KZ, B], MMDT, tag="hzpT")
                for k in range(KZ):
                    nc.sync.dma_start(hzpT[:64, k], hzp_all[2 * k, j])
                    nc.sync.dma_start(hzpT[64:128, k], hzp_all[2 * k + 1, j])
                p89 = ps_d.tile([128, B], F32, name="ps_w89", tag="psD")
                for k in range(KZ):
                    nc.tensor.matmul(p89[:], w89_s[:, k], hzpT[:, k],
                                     start=(k == 0), stop=(k == KZ - 1))
                mlp = dtmp.tile([128, B], F32, tag="mlp")
                nc.scalar.activation(mlp[:], p89[:], AF.Relu, bias=bvec[:, 2:3])
                logvp_lo = dtmp.tile([ZS, B], F32, tag="logvp_lo")
                nc.sync.dma_start(logvp_lo[:], mlp[ZS:128, :])
                stdp = dtmp.tile([ZS, B], F32, tag="stdp")
                nc.scalar.activation(stdp[:], logvp_lo[:], AF.Exp, scale=0.5)
                nzp_t = dtmp.tile([ZS, B], F32, tag="nzp_t")
                nc.sync.dma_start(nzp_t[:], nzp[j])
                nstdp = dtmp.tile([ZS, B], F32, tag="nstdp")
                nc.vector.tensor_mul(nstdp[:], nzp_t[:], stdp[:])
                zp_sl = dtmp.tile([ZS, B], F32, tag="zp_sl")
                nc.vector.tensor_add(zp_sl[:], mlp[0:ZS, :], nstdp[:])
                nc.sync.dma_start(o_zp[j], zp_sl[:])

        stack.close()
    nc.finalize()
    return nc


_NC_CACHE = {}


def _get_nc():
    if "nc" not in _NC_CACHE:
        _NC_CACHE["nc"] = _build()
    return _NC_CACHE["nc"]


def _prep_in_maps(inputs):
    f32 = np.float32
    if MMDT == F32R:
        mmnp = np.float32
    else:
        import ml_dtypes
        mmnp = ml_dtypes.bfloat16
    h_i = np.asarray(inputs["h_i"], f32)
    input_t = np.asarray(inputs["input_t"], f32)
    Wk = np.asarray(inputs["Wk"], f32)
    Wr = np.asarray(inputs["Wr"], f32)
    b_lstm = np.asarray(inputs["b_lstm"], f32)
    Ws = {n: np.asarray(inputs[n], f32) for n in
          ["W1", "W2", "W3", "W4", "W5", "W6", "W7", "W8", "W9"]}
    bs = {n: np.asarray(inputs[n], f32) for n in
          ["b1", "b2", "b3", "b4", "b5", "b6", "b7", "b8", "b9"]}
    scal = np.array([[inputs["alpha"][0, 0], inputs["beta"][0, 0],
                      inputs["base"][0, 0], inputs["tw"][0, 0]]], f32)
    noise_inf = np.asarray(inputs["noise_inf"], f32)
    noise_prior = np.asarray(inputs["noise_prior"], f32)

    hiT = np.ascontiguousarray(h_i.T)
    in_maps = []
    for r in range(NCORE):
        gcols = np.concatenate([np.arange(g * H + r * HS, g * H + (r + 1) * HS)
                                for g in range(4)])
        fcols = np.arange(r * FS, (r + 1) * FS)
        zcols = np.arange(r * ZS, (r + 1) * ZS)
        wk_sl = Wk[:, gcols]
        m = {
            "hiT": hiT,
            "t_nat": input_t,
            "scal": scal,
            "wkzy": np.ascontiguousarray(
                np.concatenate([wk_sl[0:Z], wk_sl[Z + H:]], axis=0)).astype(mmnp),
            "wkh": np.ascontiguousarray(wk_sl[Z:Z + H]),
            "wr": np.ascontiguousarray(Wr[:, gcols]).astype(mmnp),
            "blstm": np.ascontiguousarray(b_lstm[gcols].reshape(8, 128).T),
            "w1": np.ascontiguousarray(Ws["W1"][:, fcols]).astype(mmnp),
            "b1": bs["b1"][fcols].reshape(-1, 1),
            "w2": np.ascontiguousarray(Ws["W2"][:, fcols]).astype(mmnp),
            "b2": bs["b2"][fcols].reshape(-1, 1),
            "w3": np.ascontiguousarray(Ws["W3"][:, fcols]).astype(mmnp),
            "b3": bs["b3"][fcols].reshape(-1, 1),
            "w4h": np.ascontiguousarray(Ws["W4"][0:H, zcols]),
            "w4s": np.ascontiguousarray(Ws["W4"][H:, zcols]).astype(mmnp),
            "b4": bs["b4"][zcols].reshape(-1, 1),
            "w5": np.ascontiguousarray(Ws["W5"][:, zcols]).astype(mmnp),
            "b5": bs["b5"][zcols].reshape(-1, 1),
            "w6": np.ascontiguousarray(Ws["W6"][:, zcols]).astype(mmnp),
            "b6": bs["b6"][zcols].reshape(-1, 1),
            "w7h": np.ascontiguousarray(Ws["W7"][0:H, zcols]),
            "w7s": np.ascontiguousarray(Ws["W7"][H:, zcols]).astype(mmnp),
            "b7": bs["b7"][zcols].reshape(-1, 1),
            "w8": np.ascontiguousarray(Ws["W8"][:, zcols]).astype(mmnp),
            "b8": bs["b8"][zcols].reshape(-1, 1),
            "w9": np.ascontiguousarray(Ws["W9"][:, zcols]).astype(mmnp),
            "b9": bs["b9"][zcols].reshape(-1, 1),
            "nzi": np.ascontiguousarray(noise_inf[:, :, zcols].transpose(0, 2, 1)),
            "nzp": np.ascontiguousarray(noise_prior[:, :, zcols].transpose(0, 2, 1)),
        }
        in_maps.append(m)
    return in_maps


def kernel(**inputs):
    nc = _get_nc()
    in_maps = _prep_in_maps(inputs)
    res = run_bass_kernel_spmd(nc, in_maps, core_ids=list(range(NCORE)))
    results = res.results if hasattr(res, "results") else res

    def gather(name):
        parts = [results[r][name] for r in range(NCORE)]
        full = np.concatenate(parts, axis=1)        # [T, S*8, B]
        return np.ascontiguousarray(full.transpose(2, 0, 1)).astype(np.float32)

    ys = gather("o_y")
    means = gather("o_mean")
    logvs = gather("o_logv")
    zs = gather("o_z")
    zps = gather("o_zp")
    input_t_all = results[0]["o_t"].reshape(-1, 1).astype(np.float32)
    time_estimates = results[0]["o_te"].reshape(-1, 1).astype(np.float32)
    return (ys, means, logvs, zs, zps, input_t_all, time_estimates)
